# revision 17
# baseline (speedup 1.0000x reference)
"""GCN layer (message passing) on 8 trn2 NeuronCores.

  out = relu(segment_sum(norm * (H@W.T + b)[col], row)),  norm = d^-1/2[row] d^-1/2[col]
  with self-loops appended; d = 1 + in-degree.

v2 strategy (SPMD over 8 cores, nodes sharded by destination):
  - Host: pad N to 100352 = 8*12544; per dest core, edges binned by
    (dest block of 128 nodes, source bank of 25088 table rows). Bin capacity
    = max bin load over the 8 cores (exact, no pow-2 rounding) -> identical
    static layout on every core, ~10% slot padding instead of 2x.
  - Chunks are 128-slot gather columns; a chunk crossing a bin boundary gets
    one matmul per overlapped bin ("straddle"), with dk=-1 masking slots
    outside the bin. Scatter-add = S^T @ G matmul into per-block PSUM accs.
  - Device phase 1: Hl2 = (H @ W.T + b) * d^-1/2 in bf16. Bias preloaded into
    PSUM via a K=1 matmul (PE), dis-scaling via one wide DVE tensor_tensor
    with 0-stride broadcast per 4 blocks.
  - Per-half AllGather of the bf16 shard into a Shared DRAM table.
  - Phase 3 two staged passes for collective overlap: stage A consumes half-0
    banks for ALL super-blocks (overlaps the half-1 AllGather), spills bf16
    partials; stage B re-initializes PSUM via identity matmul, consumes
    half-1 banks, adds the self-loop term via identity matmul, and applies
    relu((acc) * d^-1/2[dst]) on ACT straight out of PSUM.
  - S matrices built 8-at-a-time with one DVE tensor_tensor is_equal against
    a broadcast dk column (halves DVE time vs per-chunk builds).
"""
import numpy as np

N = 100000
D = 128
NCORES = 8
P = 128
NPAD = 100352            # 8 * 12544, also 4 * 25088
NPC = NPAD // NCORES     # 12544 nodes per core
NBLK = NPC // P          # 98 dest blocks per core
NBANKS = 4
BANK = NPAD // NBANKS    # 25088 rows per bank (< 2^15 for int16 idx)
HALF = NPC // 2
HBLK = NBLK // 2
SBB = 8                  # dest blocks per super-block
NSB = (NBLK + SBB - 1) // SBB
WIDE = 8                 # S-matrices built per DVE op
GRP = 7                  # phase-1 block group (shard store batching)

KDTYPE = "bf16"
_STAGED = True           # two-pass phase 3 (overlap the 2nd AllGather)
_TIMING_SINGLE = False   # single-core build for TimelineSim (no collective)
_SHARED_AG = True        # AllGather output in Shared addr space


# ----------------------------------------------------------------- host prep

def _layout_from_caps(caps):
    """Static per-core-identical layout. caps: [NBLK, NBANKS] int array.

    Returns dict with, per (sb, k): column count; and per stage: record
    lists. Record = (sb, k, j, t, lo, hi, start, stop) where j is the chunk
    column within the (sb, k) gather and [lo, hi) the covered slot range
    within that chunk; start/stop are PSUM accumulation flags (stage A).
    Stage B start/stop are assigned on-device (identity matmuls bracket).
    """
    cols = {}
    off = {}          # (t, k) -> slot offset within its (sb, k) region
    for sb in range(NSB):
        ts = range(sb * SBB, min((sb + 1) * SBB, NBLK))
        for k in range(NBANKS):
            cur = 0
            for t in ts:
                off[(t, k)] = cur
                cur += int(caps[t, k])
            cols[(sb, k)] = (cur + P - 1) // P

    def stage_records(sb, banks):
        # grouped per destination block: PSUM accumulation chains must be
        # sequential within a bank (start=True clears whole-bank accumulate
        # bits), so all of one block's records run before the next block's
        ts = list(range(sb * SBB, min((sb + 1) * SBB, NBLK)))
        recs = []
        for t in ts:
            for k in banks:
                b0 = off[(t, k)]
                b1 = b0 + int(caps[t, k])
                for j in range(b0 // P, (b1 + P - 1) // P):
                    c0, c1 = j * P, (j + 1) * P
                    lo, hi = max(b0, c0), min(b1, c1)
                    if lo < hi:
                        recs.append([sb, k, j, t, lo - c0, hi - c0])
        return recs

    recsA = {sb: stage_records(sb, (0, 1)) for sb in range(NSB)}
    recsB = {sb: stage_records(sb, (2, 3)) for sb in range(NSB)}
    hasA = [False] * NBLK
    hasB = [False] * NBLK
    for sb in range(NSB):
        first, last = {}, {}
        for i, r in enumerate(recsA[sb]):
            t = r[3]
            hasA[t] = True
            first.setdefault(t, i)
            last[t] = i
        for i, r in enumerate(recsA[sb]):
            t = r[3]
            r.append(first[t] == i)   # start
            r.append(last[t] == i)    # stop (read by stage-A spill)
        firstB = {}
        for i, r in enumerate(recsB[sb]):
            t = r[3]
            hasB[t] = True
            firstB.setdefault(t, i)
        for i, r in enumerate(recsB[sb]):
            t = r[3]
            # staged mode: blocks with a stage-A partial are initialized by an
            # identity matmul; otherwise the first stage-B record starts.
            r.append((not hasA[t]) and firstB[t] == i)
            r.append(False)           # identity(hl2own) closes accumulation
    nrec = sum(len(recsA[sb]) + len(recsB[sb]) for sb in range(NSB))

    # idx region start per (sb, k) in gather-issue order (stage A then B)
    reg_start = {}
    cur = 0
    for sb in range(NSB):
        for k in (0, 1):
            reg_start[(sb, k)] = cur
            cur += cols[(sb, k)] * P
    for sb in range(NSB):
        for k in (2, 3):
            reg_start[(sb, k)] = cur
            cur += cols[(sb, k)] * P
    return dict(cols=cols, off=off, recsA=recsA, recsB=recsB, hasA=hasA,
                hasB=hasB, nrec=nrec, reg_start=reg_start, nslots=cur)


def _host_prep(H, edge_index, W, b):
    """Build the shared static layout + per-core device inputs."""
    import ml_dtypes
    f32 = np.float32
    bf16 = ml_dtypes.bfloat16
    row = np.asarray(edge_index[0], dtype=np.int64)
    col = np.asarray(edge_index[1], dtype=np.int64)
    H = np.asarray(H, dtype=f32)
    W = np.asarray(W, dtype=f32)
    b = np.asarray(b, dtype=f32)

    deg = (1.0 + np.bincount(row, minlength=NPAD)).astype(f32)

    Hpad = np.zeros((NPAD, D), dtype=f32)
    Hpad[:N] = H

    core = row // NPC
    block = (row % NPC) // P
    dk_all = (row % NPC) % P
    c_src = col // NPC
    r_src = col % NPC
    rr = r_src % HALF
    bank = 2 * (r_src // HALF) + (c_src // 4)
    lidx = (c_src % 4) * HALF + (rr % P) * HBLK + rr // P

    load = np.zeros((NCORES, NBLK, NBANKS), dtype=np.int64)
    np.add.at(load, (core, block, bank), 1)
    caps = load.max(axis=0)                      # [NBLK, NBANKS]
    lay = _layout_from_caps(caps)
    cols, off = lay["cols"], lay["off"]

    # rank of each edge within its (core, block, bank) bin
    order = np.lexsort((col, bank, block, core))
    sc, sb_, sk = core[order], block[order], bank[order]
    gid = (sc * NBLK + sb_) * NBANKS + sk
    starts = np.zeros(NCORES * NBLK * NBANKS, dtype=np.int64)
    np.cumsum(load.reshape(-1)[:-1], out=starts[1:])
    rank = np.arange(len(order)) - starts[gid]

    reg_start = lay["reg_start"]
    nslots = lay["nslots"]

    # absolute slot of each edge
    t_of, k_of = sb_, sk
    sb_of = t_of // SBB
    abs_slot = np.empty(len(order), dtype=np.int64)
    rs = np.array([[reg_start[(s, k)] for k in range(NBANKS)]
                   for s in range(NSB)], dtype=np.int64)
    offs = np.array([[off[(t, k)] for k in range(NBANKS)]
                     for t in range(NBLK)], dtype=np.int64)
    abs_slot = rs[sb_of, k_of] + offs[t_of, k_of] + rank

    # idx stream + dk columns per record
    idx_all = np.zeros((NCORES, nslots), dtype=np.int64)
    idx_all[sc, abs_slot] = lidx[order]

    # record list in global issue order
    all_recs = []
    for sb in range(NSB):
        all_recs.extend(lay["recsA"][sb])
    for sb in range(NSB):
        all_recs.extend(lay["recsB"][sb])
    nrec = len(all_recs)

    dkT = np.full((NCORES, P, nrec), -1.0, dtype=f32)
    # slot -> record id mapping: for each record, slots [rec_chunk_base+lo, +hi)
    # edges at those slots belong to it. Build via per-(t,k) chunk math:
    # record for edge = determined by (sb,k,j) of its slot AND t match. Since
    # bins are contiguous and records enumerate (j, t) overlaps, an edge's
    # record is unique: find record index by (t, k, j).
    rec_id = {}
    for i, r in enumerate(all_recs):
        sb, k, j, t = r[0], r[1], r[2], r[3]
        rec_id[(t, k, j)] = i
    slot_in_reg = abs_slot - rs[sb_of, k_of]
    j_of = slot_in_reg // P
    p_of = slot_in_reg % P
    rec_of = np.array([rec_id[(int(t), int(k), int(j))]
                       for t, k, j in zip(t_of, k_of, j_of)], dtype=np.int64)
    dkT[sc, p_of, rec_of] = dk_all[order]

    # idx16: wrap 16 partitions, replicate x8
    w16 = idx_all.reshape(NCORES, -1, 16).transpose(0, 2, 1)  # [c, 16, ns/16]
    idx16 = np.tile(w16, (1, 8, 1)).astype(np.int16)

    degT = np.ascontiguousarray(
        deg.reshape(NCORES, NBLK, P).transpose(0, 2, 1)).astype(f32)

    WTb = np.ascontiguousarray(W.T).astype(bf16)          # [in, out]
    bias1 = b[None, :].astype(bf16)                       # [1, D]
    ones1 = np.ones((1, P), dtype=bf16)
    ident = np.eye(P, dtype=bf16)
    iota_w = np.tile(np.arange(P, dtype=bf16)[None, :], (P, WIDE))

    in_maps = []
    for c in range(NCORES):
        HT = np.ascontiguousarray(Hpad[c * NPC:(c + 1) * NPC].T).astype(bf16)
        in_maps.append(dict(
            HT=HT,
            WTb=WTb,
            bias1=bias1,
            ones1=ones1,
            ident=ident,
            iota_w=iota_w,
            degT=np.ascontiguousarray(degT[c]),
            dkT=np.ascontiguousarray(dkT[c]),
            idx16=np.ascontiguousarray(idx16[c]),
        ))
    meta = dict(lay=lay, nslots=nslots, nrec=nrec, caps=caps)
    return in_maps, meta


# ------------------------------------------------------------- numpy device sim

def _sim_spmd(in_maps, meta):
    """Numpy mirror of the device program (index-plumbing validation)."""
    import ml_dtypes
    f32 = np.float32
    bf16 = ml_dtypes.bfloat16
    lay = meta["lay"]
    cols = lay["cols"]

    shards_h = {0: [], 1: []}
    hl2own_all = []
    dis_all = []
    for c in range(NCORES):
        m = in_maps[c]
        dis = 1.0 / np.sqrt(m["degT"])                       # [p, t]
        HT = m["HT"].astype(f32)
        WTb = m["WTb"].astype(f32)
        bias = m["bias1"].astype(f32)[0]
        hl2own = np.zeros((P, NBLK, D), dtype=f32)
        for t in range(NBLK):
            hl = HT[:, t * P:(t + 1) * P].T @ WTb + bias
            hl2own[:, t, :] = hl * dis[:, t:t + 1]
        hl2own = hl2own.astype(bf16).astype(f32)   # stg tiles are bf16
        hl2own_all.append(hl2own)
        dis_all.append(dis)
        for h in range(2):
            sl = hl2own[:, h * HBLK:(h + 1) * HBLK, :]
            shards_h[h].append(sl.reshape(HALF, D).astype(bf16))

    table_h = [np.concatenate(shards_h[h], axis=0) for h in range(2)]

    outs = []
    for c in range(NCORES):
        m = in_maps[c]
        dis = dis_all[c]
        dkT = m["dkT"].astype(f32)
        idx16 = m["idx16"]
        # unwrap idx stream
        idx_all = idx16[:16].transpose(1, 0).reshape(-1).astype(np.int64)

        # gather tiles per (sb, k) in issue order
        cursor = 0
        G = {}
        for stage, banks in ((0, (0, 1)), (1, (2, 3))):
            for sb in range(NSB):
                for k in banks:
                    n = cols[(sb, k)] * P
                    idxs = idx_all[cursor:cursor + n]
                    cursor += n
                    h, gg = k // 2, k % 2
                    tbl = table_h[h][gg * BANK:(gg + 1) * BANK]
                    G[(sb, k)] = tbl[idxs].reshape(cols[(sb, k)], P, D)

        out_c = np.zeros((NPC, D), dtype=f32)
        rec_base = 0
        accs = {}
        partA = {}
        # stage A
        for sb in range(NSB):
            recs = lay["recsA"][sb]
            for i, (sb_, k, j, t, lo, hi, st, sp) in enumerate(recs):
                ri = rec_base + i
                dk = dkT[:, ri]
                S = (np.arange(P)[None, :] == dk[:, None]).astype(f32)
                g = G[(sb, k)][j].astype(f32)     # [P slots, D]
                contrib = S.T @ g
                if st:
                    accs[t] = contrib
                else:
                    accs[t] = accs[t] + contrib
            rec_base += len(recs)
            for t in range(sb * SBB, min((sb + 1) * SBB, NBLK)):
                if lay["hasA"][t]:
                    partA[t] = accs[t].astype(bf16).astype(f32)
        # stage B
        for sb in range(NSB):
            recs = lay["recsB"][sb]
            ts = range(sb * SBB, min((sb + 1) * SBB, NBLK))
            for t in ts:
                accs[t] = partA[t].copy() if lay["hasA"][t] else np.zeros(
                    (P, D), f32)
            for i, (sb_, k, j, t, lo, hi, st, sp) in enumerate(recs):
                ri = rec_base + i
                dk = dkT[:, ri]
                S = (np.arange(P)[None, :] == dk[:, None]).astype(f32)
                g = G[(sb, k)][j].astype(f32)
                accs[t] = accs[t] + S.T @ g
            rec_base += len(recs)
            for t in ts:
                acc = accs[t] + hl2own_all[c][:, t, :]
                res = np.maximum(acc * dis[:, t:t + 1], 0.0)
                out_c[t * P:(t + 1) * P] = res.astype(bf16).astype(f32)
        outs.append(out_c)
    return np.concatenate(outs, axis=0)[:N]


# ------------------------------------------------------------- device kernel

_NC_CACHE = {}


def _build_nc(meta):
    import concourse.bacc as bacc
    import concourse.mybir as mybir
    import concourse.tile as tile
    from concourse import library_config

    lay = meta["lay"]
    cols = lay["cols"]
    nrec = meta["nrec"]
    nslots = meta["nslots"]
    kdt = mybir.dt.bfloat16
    f32 = mybir.dt.float32

    nc = bacc.Bacc("TRN2", target_bir_lowering=False, debug=False,
                   num_devices=1 if _TIMING_SINGLE else NCORES)

    HT = nc.dram_tensor("HT", [D, NPC], kdt, kind="ExternalInput").ap()
    WTb = nc.dram_tensor("WTb", [D, D], kdt, kind="ExternalInput").ap()
    bias1 = nc.dram_tensor("bias1", [1, D], kdt, kind="ExternalInput").ap()
    ones1 = nc.dram_tensor("ones1", [1, P], kdt, kind="ExternalInput").ap()
    ident = nc.dram_tensor("ident", [P, P], kdt, kind="ExternalInput").ap()
    iota_w = nc.dram_tensor("iota_w", [P, WIDE * P], kdt,
                            kind="ExternalInput").ap()
    degT = nc.dram_tensor("degT", [P, NBLK], f32, kind="ExternalInput").ap()
    dkT = nc.dram_tensor("dkT", [P, nrec], f32, kind="ExternalInput").ap()
    idx16 = nc.dram_tensor("idx16", [P, nslots // 16], mybir.dt.int16,
                           kind="ExternalInput").ap()
    out = nc.dram_tensor("out", [P, NBLK * D], kdt, kind="ExternalOutput").ap()

    with tile.TileContext(nc) as tc:
        with (
            tc.tile_pool(name="const", bufs=1) as const,
            tc.tile_pool(name="big", bufs=1) as big,
            tc.tile_pool(name="dram", bufs=1, space="DRAM") as dram,
        ):
            nc.gpsimd.load_library(library_config.mlp)

            WTb_s = const.tile([D, D], kdt)
            nc.sync.dma_start(out=WTb_s[:], in_=WTb[:])
            bias1_s = const.tile([1, D], kdt)
            nc.sync.dma_start(out=bias1_s[:], in_=bias1[:])
            ones1_s = const.tile([1, P], kdt)
            nc.sync.dma_start(out=ones1_s[:], in_=ones1[:])
            ident_s = const.tile([P, P], kdt)
            nc.sync.dma_start(out=ident_s[:], in_=ident[:])
            iota_s = const.tile([P, WIDE * P], kdt)
            nc.sync.dma_start(out=iota_s[:], in_=iota_w[:])
            degT_s = const.tile([P, NBLK], f32)
            nc.sync.dma_start(out=degT_s[:], in_=degT[:])

            rec_s = const.tile([P, NBLK], f32)
            nc.vector.reciprocal(out=rec_s[:], in_=degT_s[:])
            disT_s = const.tile([P, NBLK], f32)
            nc.scalar.sqrt(out=disT_s[:], in_=rec_s[:])

            dkT_s = big.tile([P, nrec], f32)
            nc.scalar.dma_start(out=dkT_s[:], in_=dkT[:])
            idx_s = big.tile([P, nslots // 16], mybir.dt.int16)
            nc.scalar.dma_start(out=idx_s[:], in_=idx16[:])

            shard_h = [dram.tile([HALF, D], kdt, name=f"shard_h{h}")
                       for h in range(2)]
            table_h = [dram.tile([NCORES * HALF, D], kdt, name=f"table_h{h}",
                                 addr_space=("Shared" if (_SHARED_AG and
                                             not _TIMING_SINGLE) else "Local"))
                       for h in range(2)]

            # ---------------- phase 1 + per-half AllGather
            hl2own_s = {}
            with (
                tc.tile_pool(name="p1psum", bufs=2, space="PSUM") as p1psum,
                tc.tile_pool(name="p1ht", bufs=3) as p1ht,
            ):
                for h in range(2):
                    for g0 in range(h * HBLK, (h + 1) * HBLK, GRP):
                        gn = min(GRP, (h + 1) * HBLK - g0)
                        eng = nc.sync if (g0 // GRP) % 2 == 0 else nc.scalar
                        ht = p1ht.tile([D, GRP * P], kdt, tag="ht")
                        eng.dma_start(out=ht[:, :gn * P],
                                      in_=HT[:, g0 * P:(g0 + gn) * P])
                        stg = big.tile([P, GRP * D], kdt, name=f"stg_{g0}")
                        hl2own_s[g0 // GRP] = stg
                        for q0 in range(0, gn, 4):
                            qn = min(4, gn - q0)
                            ps = p1psum.tile([P, 4 * D], f32, space="PSUM",
                                             tag="ps")
                            for lt in range(q0, q0 + qn):
                                sl = lt - q0
                                nc.tensor.matmul(
                                    out=ps[:, sl * D:(sl + 1) * D],
                                    lhsT=ones1_s[:], rhs=bias1_s[:],
                                    start=True, stop=False)
                                nc.tensor.matmul(
                                    out=ps[:, sl * D:(sl + 1) * D],
                                    lhsT=ht[:, lt * P:(lt + 1) * P],
                                    rhs=WTb_s[:], start=False, stop=True)
                            bc = disT_s[:, g0 + q0:g0 + q0 + qn].unsqueeze(
                                2).broadcast_to([P, qn, D])
                            nc.vector.tensor_tensor(
                                out=stg[:, q0 * D:(q0 + qn) * D],
                                in0=ps[:, :qn * D], in1=bc,
                                op=mybir.AluOpType.mult)
                        lb = g0 - h * HBLK
                        eng.dma_start(
                            out=shard_h[h][:].rearrange(
                                "(p l) f -> p (l f)", p=P)[:, lb * D:(lb + gn) * D],
                            in_=stg[:, :gn * D])
                    if _TIMING_SINGLE:
                        for c in range(NCORES):
                            nc.gpsimd.dma_start(
                                out=table_h[h][c * HALF:(c + 1) * HALF, :],
                                in_=shard_h[h][:])
                    else:
                        nc.gpsimd.collective_compute(
                            "AllGather", mybir.AluOpType.bypass,
                            replica_groups=[list(range(NCORES))],
                            ins=[shard_h[h].opt()],
                            outs=[table_h[h].opt()],
                        )

            # ---------------- phase 3: edge aggregation
            _run_phase3(nc, tc, mybir, tile, lay, kdt, f32, table_h,
                        dkT_s, idx_s, iota_s, ident_s, disT_s, hl2own_s, out,
                        big)

    nc.finalize()
    return nc


def _run_phase3(nc, tc, mybir, tile, lay, kdt, f32, table_h, dkT_s, idx_s,
                iota_s, ident_s, disT_s, hl2own_s, out, big):
    cols = lay["cols"]
    hasA = lay["hasA"]
    hasB = lay["hasB"]
    reg_start = lay["reg_start"]
    COLMAX = max(cols.values())

    def hl2_slice(t):
        return hl2own_s[t // GRP][:, (t % GRP) * D:(t % GRP + 1) * D]

    partA = big.tile([P, NBLK * D], kdt, name="partA") if _STAGED else None

    with (
        tc.tile_pool(name="gpool", bufs=8) as gpool,
        tc.tile_pool(name="spool", bufs=4) as spool,
        tc.tile_pool(name="acc", bufs=4, space="PSUM") as accp,
        tc.tile_pool(name="epi", bufs=4) as epi,
    ):
        def alloc_accs(sb, ts, tagpfx):
            # 4 block-accumulators per PSUM bank tile; consecutive blocks
            # alternate between 2 bank tiles so ACT spills of one bank overlap
            # PE chains on the other. Chains within a bank stay sequential.
            accs = {}
            nb = min(2, len(ts))
            bankts = [accp.tile([P, 4 * D], f32, space="PSUM", tag="acc",
                                name=f"{tagpfx}_{sb}_{i}") for i in range(nb)]
            for i, t in enumerate(ts):
                accs[t] = bankts[i % nb][:, (i // nb) * D:(i // nb + 1) * D]
            return accs

        GCAP = 1024 // P     # max gather columns per dma_gather call

        def do_gather(sb, k):
            ncol = cols[(sb, k)]
            if ncol == 0:
                return None
            g = gpool.tile([P, COLMAX, D], kdt, tag=f"g{k % 2}",
                           name=f"g_{sb}_{k}")
            h, gg = k // 2, k % 2
            c0 = reg_start[(sb, k)] // 16
            for p0 in range(0, ncol, GCAP):
                pn = min(GCAP, ncol - p0)
                nidx = pn * P
                nc.gpsimd.dma_gather(
                    g[:, p0:p0 + pn, :],
                    table_h[h][gg * BANK:(gg + 1) * BANK, :],
                    idx_s[:, c0 + p0 * 8:c0 + p0 * 8 + nidx // 16],
                    nidx, nidx, D)
            return g

        def do_record(ri, k, j, t, st, sp, G, accs):
            # per-record S build: tensor_scalar is_equal hits the DVE 4x
            # fast mode (a wide tensor_tensor with a 0-stride broadcast
            # operand does not)
            S = spool.tile([P, P], kdt, tag="s")
            nc.vector.tensor_scalar(
                out=S[:], in0=iota_s[:, :P],
                scalar1=dkT_s[:, ri:ri + 1],
                scalar2=None, op0=mybir.AluOpType.is_equal)
            nc.tensor.matmul(
                out=accs[t], lhsT=S[:],
                rhs=G[k][:, j, :], start=st, stop=sp)

        def do_records(recs, G, accs, rec_base):
            for i, rec in enumerate(recs):
                _sb, k, j, t, lo, hi, st, sp = rec
                do_record(rec_base + i, k, j, t, st, sp, G, accs)

        nA = sum(len(lay["recsA"][s]) for s in range(NSB))
        if _STAGED:
            # ---- stage A: banks 0,1 (half 0) for every super-block
            rec_base = 0
            for sb in range(NSB):
                ts = list(range(sb * SBB, min((sb + 1) * SBB, NBLK)))
                G = {k: do_gather(sb, k) for k in (0, 1)}
                accs = alloc_accs(sb, ts, "accA")
                recs = lay["recsA"][sb]
                do_records(recs, G, accs, rec_base)
                rec_base += len(recs)
                for t in ts:
                    if hasA[t]:
                        nc.scalar.activation(
                            out=partA[:, t * D:(t + 1) * D], in_=accs[t],
                            func=mybir.ActivationFunctionType.Copy)
            # ---- stage B: banks 2,3 (half 1) + self-loop + epilogue.
            # Fully sequential chain per block within its PSUM bank:
            # init(partA) -> records -> identity(hl2own, stop).
            rec_base = nA
            for sb in range(NSB):
                ts = list(range(sb * SBB, min((sb + 1) * SBB, NBLK)))
                G = {k: do_gather(sb, k) for k in (2, 3)}
                accs = alloc_accs(sb, ts, "accB")
                recs = lay["recsB"][sb]
                by_t = {}
                for i, rec in enumerate(recs):
                    by_t.setdefault(rec[3], []).append((rec_base + i, rec))
                for t in ts:
                    if hasA[t]:
                        nc.tensor.matmul(out=accs[t], lhsT=ident_s[:],
                                         rhs=partA[:, t * D:(t + 1) * D],
                                         start=True, stop=False)
                    for ri, rec in by_t.get(t, []):
                        _sb, k, j, _t, lo, hi, st, sp = rec
                        do_record(ri, k, j, t, st, sp, G, accs)
                    nc.tensor.matmul(out=accs[t], lhsT=ident_s[:],
                                     rhs=hl2_slice(t),
                                     start=not hasA[t] and not hasB[t],
                                     stop=True)
                rec_base += len(recs)
                ostg = epi.tile([P, SBB * D], kdt, tag="ostg")
                for lt, t in enumerate(ts):
                    nc.scalar.activation(
                        out=ostg[:, lt * D:(lt + 1) * D], in_=accs[t],
                        func=mybir.ActivationFunctionType.Relu,
                        scale=disT_s[:, t:t + 1])
                eng = nc.sync if sb % 2 == 0 else nc.scalar
                eng.dma_start(
                    out=out[:, sb * SBB * D:(sb * SBB + len(ts)) * D],
                    in_=ostg[:, :len(ts) * D])
        else:
            # single pass per super-block; one continuous chain per block:
            # records(A) -> records(B) -> identity(hl2own, stop)
            baseA = 0
            baseB = nA
            for sb in range(NSB):
                ts = list(range(sb * SBB, min((sb + 1) * SBB, NBLK)))
                G = {k: do_gather(sb, k) for k in range(NBANKS)}
                accs = alloc_accs(sb, ts, "acc")
                recsA = lay["recsA"][sb]
                recsB = lay["recsB"][sb]
                by_t = {}
                for i, rec in enumerate(recsA):
                    by_t.setdefault(rec[3], []).append((baseA + i, rec))
                for i, rec in enumerate(recsB):
                    by_t.setdefault(rec[3], []).append((baseB + i, rec))
                for t in ts:
                    first = True
                    for ri, rec in by_t.get(t, []):
                        _sb, k, j, _t, lo, hi, _st, _sp = rec
                        do_record(ri, k, j, t, first, False, G, accs)
                        first = False
                    nc.tensor.matmul(out=accs[t], lhsT=ident_s[:],
                                     rhs=hl2_slice(t),
                                     start=first, stop=True)
                baseA += len(recsA)
                baseB += len(recsB)
                ostg = epi.tile([P, SBB * D], kdt, tag="ostg")
                for lt, t in enumerate(ts):
                    nc.scalar.activation(
                        out=ostg[:, lt * D:(lt + 1) * D], in_=accs[t],
                        func=mybir.ActivationFunctionType.Relu,
                        scale=disT_s[:, t:t + 1])
                eng = nc.sync if sb % 2 == 0 else nc.scalar
                eng.dma_start(
                    out=out[:, sb * SBB * D:(sb * SBB + len(ts)) * D],
                    in_=ostg[:, :len(ts) * D])


_PB_USED = None  # legacy hook for bench.py; now caches layout key


def kernel(H, edge_index, W, b):
    from concourse.bass_utils import run_bass_kernel_spmd

    global _PB_USED
    in_maps, meta = _host_prep(H, edge_index, W, b)
    key = (meta["nrec"], meta["nslots"], _STAGED, _SHARED_AG)
    _PB_USED = key
    if key not in _NC_CACHE:
        _NC_CACHE[key] = _build_nc(meta)
    nc = _NC_CACHE[key]

    res = run_bass_kernel_spmd(nc, in_maps, list(range(NCORES)))
    outs = []
    for c in range(NCORES):
        o = np.asarray(res.results[c]["out"]).reshape(P, NBLK, D)
        outs.append(o.transpose(1, 0, 2).reshape(NPC, D).astype(np.float32))
    out = np.concatenate(outs, axis=0)
    return np.ascontiguousarray(out[:N])


# revision 18
# speedup vs baseline: 2.0114x; 2.0114x over previous
"""GCN layer (message passing) on 8 trn2 NeuronCores.

  out = relu(segment_sum(norm * (H@W.T + b)[col], row)),  norm = d^-1/2[row] d^-1/2[col]
  with self-loops appended; d = 1 + in-degree.

v2 strategy (SPMD over 8 cores, nodes sharded by destination):
  - Host: pad N to 100352 = 8*12544; per dest core, edges binned by
    (dest block of 128 nodes, source bank of 25088 table rows). Bin capacity
    = max bin load over the 8 cores (exact, no pow-2 rounding) -> identical
    static layout on every core, ~10% slot padding instead of 2x.
  - Chunks are 128-slot gather columns; a chunk crossing a bin boundary gets
    one matmul per overlapped bin ("straddle"), with dk=-1 masking slots
    outside the bin. Scatter-add = S^T @ G matmul into per-block PSUM accs.
  - Device phase 1: Hl2 = (H @ W.T + b) * d^-1/2 in bf16. Bias preloaded into
    PSUM via a K=1 matmul (PE), dis-scaling via one wide DVE tensor_tensor
    with 0-stride broadcast per 4 blocks.
  - Per-half AllGather of the bf16 shard into a Shared DRAM table.
  - Phase 3 two staged passes for collective overlap: stage A consumes half-0
    banks for ALL super-blocks (overlaps the half-1 AllGather), spills bf16
    partials; stage B re-initializes PSUM via identity matmul, consumes
    half-1 banks, adds the self-loop term via identity matmul, and applies
    relu((acc) * d^-1/2[dst]) on ACT straight out of PSUM.
  - S matrices built 8-at-a-time with one DVE tensor_tensor is_equal against
    a broadcast dk column (halves DVE time vs per-chunk builds).
"""
import numpy as np

N = 100000
D = 128
NCORES = 8
P = 128
NPAD = 100352            # 8 * 12544, also 4 * 25088
NPC = NPAD // NCORES     # 12544 nodes per core
NBLK = NPC // P          # 98 dest blocks per core
NBANKS = 4
BANK = NPAD // NBANKS    # 25088 rows per bank (< 2^15 for int16 idx)
HALF = NPC // 2
HBLK = NBLK // 2
SBB = 8                  # dest blocks per super-block
NSB = (NBLK + SBB - 1) // SBB
WIDE = 8                 # S-matrices built per DVE op
GRP = 7                  # phase-1 block group (shard store batching)

KDTYPE = "bf16"
_STAGED = True           # two-pass phase 3 (overlap the 2nd AllGather)
_TIMING_SINGLE = False   # single-core build for TimelineSim (no collective)
_SHARED_AG = True        # AllGather output in Shared addr space


# ----------------------------------------------------------------- host prep

def _layout_from_caps(caps):
    """Static per-core-identical layout. caps: [NBLK, NBANKS] int array.

    Returns dict with, per (sb, k): column count; and per stage: record
    lists. Record = (sb, k, j, t, lo, hi, start, stop) where j is the chunk
    column within the (sb, k) gather and [lo, hi) the covered slot range
    within that chunk; start/stop are PSUM accumulation flags (stage A).
    Stage B start/stop are assigned on-device (identity matmuls bracket).
    """
    cols = {}
    off = {}          # (t, k) -> slot offset within its (sb, k) region
    for sb in range(NSB):
        ts = range(sb * SBB, min((sb + 1) * SBB, NBLK))
        for k in range(NBANKS):
            cur = 0
            for t in ts:
                off[(t, k)] = cur
                cur += int(caps[t, k])
            cols[(sb, k)] = (cur + P - 1) // P

    def stage_records(sb, banks):
        # grouped per destination block: PSUM accumulation chains must be
        # sequential within a bank (start=True clears whole-bank accumulate
        # bits), so all of one block's records run before the next block's
        ts = list(range(sb * SBB, min((sb + 1) * SBB, NBLK)))
        recs = []
        for t in ts:
            for k in banks:
                b0 = off[(t, k)]
                b1 = b0 + int(caps[t, k])
                for j in range(b0 // P, (b1 + P - 1) // P):
                    c0, c1 = j * P, (j + 1) * P
                    lo, hi = max(b0, c0), min(b1, c1)
                    if lo < hi:
                        recs.append([sb, k, j, t, lo - c0, hi - c0])
        return recs

    recsA = {sb: stage_records(sb, (0, 1)) for sb in range(NSB)}
    recsB = {sb: stage_records(sb, (2, 3)) for sb in range(NSB)}
    hasA = [False] * NBLK
    hasB = [False] * NBLK
    for sb in range(NSB):
        first, last = {}, {}
        for i, r in enumerate(recsA[sb]):
            t = r[3]
            hasA[t] = True
            first.setdefault(t, i)
            last[t] = i
        for i, r in enumerate(recsA[sb]):
            t = r[3]
            r.append(first[t] == i)   # start
            r.append(last[t] == i)    # stop (read by stage-A spill)
        firstB = {}
        for i, r in enumerate(recsB[sb]):
            t = r[3]
            hasB[t] = True
            firstB.setdefault(t, i)
        for i, r in enumerate(recsB[sb]):
            t = r[3]
            # staged mode: blocks with a stage-A partial are initialized by an
            # identity matmul; otherwise the first stage-B record starts.
            r.append((not hasA[t]) and firstB[t] == i)
            r.append(False)           # identity(hl2own) closes accumulation
    nrec = sum(len(recsA[sb]) + len(recsB[sb]) for sb in range(NSB))

    # idx region start per (sb, k) in gather-issue order (stage A then B)
    reg_start = {}
    cur = 0
    for sb in range(NSB):
        for k in (0, 1):
            reg_start[(sb, k)] = cur
            cur += cols[(sb, k)] * P
    for sb in range(NSB):
        for k in (2, 3):
            reg_start[(sb, k)] = cur
            cur += cols[(sb, k)] * P
    return dict(cols=cols, off=off, recsA=recsA, recsB=recsB, hasA=hasA,
                hasB=hasB, nrec=nrec, reg_start=reg_start, nslots=cur)


def _host_prep(H, edge_index, W, b):
    """Build the shared static layout + per-core device inputs."""
    import ml_dtypes
    f32 = np.float32
    bf16 = ml_dtypes.bfloat16
    row = np.asarray(edge_index[0], dtype=np.int64)
    col = np.asarray(edge_index[1], dtype=np.int64)
    H = np.asarray(H, dtype=f32)
    W = np.asarray(W, dtype=f32)
    b = np.asarray(b, dtype=f32)

    deg = (1.0 + np.bincount(row, minlength=NPAD)).astype(f32)

    Hpad = np.zeros((NPAD, D), dtype=f32)
    Hpad[:N] = H

    core = row // NPC
    block = (row % NPC) // P
    dk_all = (row % NPC) % P
    c_src = col // NPC
    r_src = col % NPC
    rr = r_src % HALF
    bank = 2 * (r_src // HALF) + (c_src // 4)
    lidx = (c_src % 4) * HALF + (rr % P) * HBLK + rr // P

    load = np.zeros((NCORES, NBLK, NBANKS), dtype=np.int64)
    np.add.at(load, (core, block, bank), 1)
    caps = load.max(axis=0)                      # [NBLK, NBANKS]
    lay = _layout_from_caps(caps)
    cols, off = lay["cols"], lay["off"]

    # rank of each edge within its (core, block, bank) bin
    order = np.lexsort((col, bank, block, core))
    sc, sb_, sk = core[order], block[order], bank[order]
    gid = (sc * NBLK + sb_) * NBANKS + sk
    starts = np.zeros(NCORES * NBLK * NBANKS, dtype=np.int64)
    np.cumsum(load.reshape(-1)[:-1], out=starts[1:])
    rank = np.arange(len(order)) - starts[gid]

    reg_start = lay["reg_start"]
    nslots = lay["nslots"]

    # absolute slot of each edge
    t_of, k_of = sb_, sk
    sb_of = t_of // SBB
    abs_slot = np.empty(len(order), dtype=np.int64)
    rs = np.array([[reg_start[(s, k)] for k in range(NBANKS)]
                   for s in range(NSB)], dtype=np.int64)
    offs = np.array([[off[(t, k)] for k in range(NBANKS)]
                     for t in range(NBLK)], dtype=np.int64)
    abs_slot = rs[sb_of, k_of] + offs[t_of, k_of] + rank

    # idx stream + dk columns per record
    idx_all = np.zeros((NCORES, nslots), dtype=np.int64)
    idx_all[sc, abs_slot] = lidx[order]

    # record list in global issue order
    all_recs = []
    for sb in range(NSB):
        all_recs.extend(lay["recsA"][sb])
    for sb in range(NSB):
        all_recs.extend(lay["recsB"][sb])
    nrec = len(all_recs)

    dkT = np.full((NCORES, P, nrec), -1.0, dtype=f32)
    # slot -> record id mapping: for each record, slots [rec_chunk_base+lo, +hi)
    # edges at those slots belong to it. Build via per-(t,k) chunk math:
    # record for edge = determined by (sb,k,j) of its slot AND t match. Since
    # bins are contiguous and records enumerate (j, t) overlaps, an edge's
    # record is unique: find record index by (t, k, j).
    rec_id = {}
    for i, r in enumerate(all_recs):
        sb, k, j, t = r[0], r[1], r[2], r[3]
        rec_id[(t, k, j)] = i
    slot_in_reg = abs_slot - rs[sb_of, k_of]
    j_of = slot_in_reg // P
    p_of = slot_in_reg % P
    rec_of = np.array([rec_id[(int(t), int(k), int(j))]
                       for t, k, j in zip(t_of, k_of, j_of)], dtype=np.int64)
    dkT[sc, p_of, rec_of] = dk_all[order]

    # idx16: wrap 16 partitions, replicate x8
    w16 = idx_all.reshape(NCORES, -1, 16).transpose(0, 2, 1)  # [c, 16, ns/16]
    idx16 = np.tile(w16, (1, 8, 1)).astype(np.int16)

    degT = np.ascontiguousarray(
        deg.reshape(NCORES, NBLK, P).transpose(0, 2, 1)).astype(f32)

    WTb = np.ascontiguousarray(W.T).astype(bf16)          # [in, out]
    bias1 = b[None, :].astype(bf16)                       # [1, D]
    ones1 = np.ones((1, P), dtype=bf16)
    ident = np.eye(P, dtype=bf16)
    iota_w = np.tile(np.arange(P, dtype=bf16)[None, :], (P, WIDE))

    in_maps = []
    for c in range(NCORES):
        HT = np.ascontiguousarray(Hpad[c * NPC:(c + 1) * NPC].T).astype(bf16)
        in_maps.append(dict(
            HT=HT,
            WTb=WTb,
            bias1=bias1,
            ones1=ones1,
            ident=ident,
            iota_w=iota_w,
            degT=np.ascontiguousarray(degT[c]),
            dkT=np.ascontiguousarray(dkT[c]),
            idx16=np.ascontiguousarray(idx16[c]),
        ))
    meta = dict(lay=lay, nslots=nslots, nrec=nrec, caps=caps)
    return in_maps, meta


# ------------------------------------------------------------- numpy device sim

def _sim_spmd(in_maps, meta):
    """Numpy mirror of the device program (index-plumbing validation)."""
    import ml_dtypes
    f32 = np.float32
    bf16 = ml_dtypes.bfloat16
    lay = meta["lay"]
    cols = lay["cols"]

    shards_h = {0: [], 1: []}
    hl2own_all = []
    dis_all = []
    for c in range(NCORES):
        m = in_maps[c]
        dis = 1.0 / np.sqrt(m["degT"])                       # [p, t]
        HT = m["HT"].astype(f32)
        WTb = m["WTb"].astype(f32)
        bias = m["bias1"].astype(f32)[0]
        hl2own = np.zeros((P, NBLK, D), dtype=f32)
        for t in range(NBLK):
            hl = HT[:, t * P:(t + 1) * P].T @ WTb + bias
            hl2own[:, t, :] = hl * dis[:, t:t + 1]
        hl2own = hl2own.astype(bf16).astype(f32)   # stg tiles are bf16
        hl2own_all.append(hl2own)
        dis_all.append(dis)
        for h in range(2):
            sl = hl2own[:, h * HBLK:(h + 1) * HBLK, :]
            shards_h[h].append(sl.reshape(HALF, D).astype(bf16))

    table_h = [np.concatenate(shards_h[h], axis=0) for h in range(2)]

    outs = []
    for c in range(NCORES):
        m = in_maps[c]
        dis = dis_all[c]
        dkT = m["dkT"].astype(f32)
        idx16 = m["idx16"]
        # unwrap idx stream
        idx_all = idx16[:16].transpose(1, 0).reshape(-1).astype(np.int64)

        # gather tiles per (sb, k) in issue order
        cursor = 0
        G = {}
        for stage, banks in ((0, (0, 1)), (1, (2, 3))):
            for sb in range(NSB):
                for k in banks:
                    n = cols[(sb, k)] * P
                    idxs = idx_all[cursor:cursor + n]
                    cursor += n
                    h, gg = k // 2, k % 2
                    tbl = table_h[h][gg * BANK:(gg + 1) * BANK]
                    G[(sb, k)] = tbl[idxs].reshape(cols[(sb, k)], P, D)

        out_c = np.zeros((NPC, D), dtype=f32)
        rec_base = 0
        accs = {}
        partA = {}
        # stage A
        for sb in range(NSB):
            recs = lay["recsA"][sb]
            for i, (sb_, k, j, t, lo, hi, st, sp) in enumerate(recs):
                ri = rec_base + i
                dk = dkT[:, ri]
                S = (np.arange(P)[None, :] == dk[:, None]).astype(f32)
                g = G[(sb, k)][j].astype(f32)     # [P slots, D]
                contrib = S.T @ g
                if st:
                    accs[t] = contrib
                else:
                    accs[t] = accs[t] + contrib
            rec_base += len(recs)
            for t in range(sb * SBB, min((sb + 1) * SBB, NBLK)):
                if lay["hasA"][t]:
                    partA[t] = accs[t].astype(bf16).astype(f32)
        # stage B
        for sb in range(NSB):
            recs = lay["recsB"][sb]
            ts = range(sb * SBB, min((sb + 1) * SBB, NBLK))
            for t in ts:
                accs[t] = partA[t].copy() if lay["hasA"][t] else np.zeros(
                    (P, D), f32)
            for i, (sb_, k, j, t, lo, hi, st, sp) in enumerate(recs):
                ri = rec_base + i
                dk = dkT[:, ri]
                S = (np.arange(P)[None, :] == dk[:, None]).astype(f32)
                g = G[(sb, k)][j].astype(f32)
                accs[t] = accs[t] + S.T @ g
            rec_base += len(recs)
            for t in ts:
                acc = accs[t] + hl2own_all[c][:, t, :]
                res = np.maximum(acc * dis[:, t:t + 1], 0.0)
                out_c[t * P:(t + 1) * P] = res.astype(bf16).astype(f32)
        outs.append(out_c)
    return np.concatenate(outs, axis=0)[:N]


# ------------------------------------------------------------- device kernel

_NC_CACHE = {}


def _build_nc(meta):
    import concourse.bacc as bacc
    import concourse.mybir as mybir
    import concourse.tile as tile
    from concourse import library_config

    lay = meta["lay"]
    cols = lay["cols"]
    nrec = meta["nrec"]
    nslots = meta["nslots"]
    kdt = mybir.dt.bfloat16
    f32 = mybir.dt.float32

    nc = bacc.Bacc("TRN2", target_bir_lowering=False, debug=False,
                   num_devices=1 if _TIMING_SINGLE else NCORES)

    HT = nc.dram_tensor("HT", [D, NPC], kdt, kind="ExternalInput").ap()
    WTb = nc.dram_tensor("WTb", [D, D], kdt, kind="ExternalInput").ap()
    bias1 = nc.dram_tensor("bias1", [1, D], kdt, kind="ExternalInput").ap()
    ones1 = nc.dram_tensor("ones1", [1, P], kdt, kind="ExternalInput").ap()
    ident = nc.dram_tensor("ident", [P, P], kdt, kind="ExternalInput").ap()
    iota_w = nc.dram_tensor("iota_w", [P, WIDE * P], kdt,
                            kind="ExternalInput").ap()
    degT = nc.dram_tensor("degT", [P, NBLK], f32, kind="ExternalInput").ap()
    dkT = nc.dram_tensor("dkT", [P, nrec], f32, kind="ExternalInput").ap()
    idx16 = nc.dram_tensor("idx16", [P, nslots // 16], mybir.dt.int16,
                           kind="ExternalInput").ap()
    out = nc.dram_tensor("out", [P, NBLK * D], kdt, kind="ExternalOutput").ap()

    with tile.TileContext(nc) as tc:
        with (
            tc.tile_pool(name="const", bufs=1) as const,
            tc.tile_pool(name="big", bufs=1) as big,
            tc.tile_pool(name="dram", bufs=1, space="DRAM") as dram,
        ):
            nc.gpsimd.load_library(library_config.mlp)

            WTb_s = const.tile([D, D], kdt)
            nc.sync.dma_start(out=WTb_s[:], in_=WTb[:])
            bias1_s = const.tile([1, D], kdt)
            nc.sync.dma_start(out=bias1_s[:], in_=bias1[:])
            ones1_s = const.tile([1, P], kdt)
            nc.sync.dma_start(out=ones1_s[:], in_=ones1[:])
            ident_s = const.tile([P, P], kdt)
            nc.sync.dma_start(out=ident_s[:], in_=ident[:])
            iota_s = const.tile([P, WIDE * P], kdt)
            nc.sync.dma_start(out=iota_s[:], in_=iota_w[:])
            degT_s = const.tile([P, NBLK], f32)
            nc.sync.dma_start(out=degT_s[:], in_=degT[:])

            rec_s = const.tile([P, NBLK], f32)
            nc.vector.reciprocal(out=rec_s[:], in_=degT_s[:])
            disT_s = const.tile([P, NBLK], f32)
            nc.scalar.sqrt(out=disT_s[:], in_=rec_s[:])

            dkT_s = big.tile([P, nrec], f32)
            idx_s = big.tile([P, nslots // 16], mybir.dt.int16)

            shard_h = [dram.tile([HALF, D], kdt, name=f"shard_h{h}")
                       for h in range(2)]
            table_h = [dram.tile([NCORES * HALF, D], kdt, name=f"table_h{h}",
                                 addr_space=("Shared" if (_SHARED_AG and
                                             not _TIMING_SINGLE) else "Local"))
                       for h in range(2)]

            # ---------------- phase 1 + per-half AllGather
            hl2own_s = {}
            with (
                tc.tile_pool(name="p1psum", bufs=2, space="PSUM") as p1psum,
                tc.tile_pool(name="p1ht", bufs=3) as p1ht,
            ):
                for h in range(2):
                    for g0 in range(h * HBLK, (h + 1) * HBLK, GRP):
                        gn = min(GRP, (h + 1) * HBLK - g0)
                        eng = nc.sync if (g0 // GRP) % 2 == 0 else nc.scalar
                        ht = p1ht.tile([D, GRP * P], kdt, tag="ht")
                        eng.dma_start(out=ht[:, :gn * P],
                                      in_=HT[:, g0 * P:(g0 + gn) * P])
                        stg = big.tile([P, GRP * D], kdt, name=f"stg_{g0}")
                        hl2own_s[g0 // GRP] = stg
                        for q0 in range(0, gn, 4):
                            qn = min(4, gn - q0)
                            ps = p1psum.tile([P, 4 * D], f32, space="PSUM",
                                             tag="ps")
                            for lt in range(q0, q0 + qn):
                                sl = lt - q0
                                nc.tensor.matmul(
                                    out=ps[:, sl * D:(sl + 1) * D],
                                    lhsT=ones1_s[:], rhs=bias1_s[:],
                                    start=True, stop=False)
                                nc.tensor.matmul(
                                    out=ps[:, sl * D:(sl + 1) * D],
                                    lhsT=ht[:, lt * P:(lt + 1) * P],
                                    rhs=WTb_s[:], start=False, stop=True)
                            bc = disT_s[:, g0 + q0:g0 + q0 + qn].unsqueeze(
                                2).broadcast_to([P, qn, D])
                            nc.vector.tensor_tensor(
                                out=stg[:, q0 * D:(q0 + qn) * D],
                                in0=ps[:, :qn * D], in1=bc,
                                op=mybir.AluOpType.mult)
                        lb = g0 - h * HBLK
                        eng.dma_start(
                            out=shard_h[h][:].rearrange(
                                "(p l) f -> p (l f)", p=P)[:, lb * D:(lb + gn) * D],
                            in_=stg[:, :gn * D])
                    if _TIMING_SINGLE:
                        for c in range(NCORES):
                            nc.gpsimd.dma_start(
                                out=table_h[h][c * HALF:(c + 1) * HALF, :],
                                in_=shard_h[h][:])
                    else:
                        nc.gpsimd.collective_compute(
                            "AllGather", mybir.AluOpType.bypass,
                            replica_groups=[list(range(NCORES))],
                            ins=[shard_h[h].opt()],
                            outs=[table_h[h].opt()],
                        )

            # phase-3 streams load behind phase 1 (needed only by gathers)
            nc.scalar.dma_start(out=dkT_s[:], in_=dkT[:])
            nc.scalar.dma_start(out=idx_s[:], in_=idx16[:])

            # ---------------- phase 3: edge aggregation
            _run_phase3(nc, tc, mybir, tile, lay, kdt, f32, table_h,
                        dkT_s, idx_s, iota_s, ident_s, disT_s, hl2own_s, out,
                        big)

    nc.finalize()
    return nc


def _run_phase3(nc, tc, mybir, tile, lay, kdt, f32, table_h, dkT_s, idx_s,
                iota_s, ident_s, disT_s, hl2own_s, out, big):
    cols = lay["cols"]
    hasA = lay["hasA"]
    hasB = lay["hasB"]
    reg_start = lay["reg_start"]
    COLMAX = max(cols.values())

    def hl2_slice(t):
        return hl2own_s[t // GRP][:, (t % GRP) * D:(t % GRP + 1) * D]

    partA = big.tile([P, NBLK * D], kdt, name="partA") if _STAGED else None

    with (
        tc.tile_pool(name="gpool", bufs=10) as gpool,
        tc.tile_pool(name="spool", bufs=6) as spool,
        tc.tile_pool(name="acc", bufs=6, space="PSUM") as accp,
        tc.tile_pool(name="epi", bufs=4) as epi,
    ):
        def alloc_accs(sb, ts, tagpfx):
            # 4 block-accumulators per PSUM bank tile; consecutive blocks
            # alternate between 2 bank tiles so ACT spills of one bank overlap
            # PE chains on the other. Chains within a bank stay sequential.
            accs = {}
            nb = min(2, len(ts))
            bankts = [accp.tile([P, 4 * D], f32, space="PSUM", tag="acc",
                                name=f"{tagpfx}_{sb}_{i}") for i in range(nb)]
            for i, t in enumerate(ts):
                accs[t] = bankts[i % nb][:, (i // nb) * D:(i // nb + 1) * D]
            return accs

        GCAP = 1024 // P     # max gather columns per dma_gather call

        def do_gather(sb, k):
            ncol = cols[(sb, k)]
            if ncol == 0:
                return None
            g = gpool.tile([P, COLMAX, D], kdt, tag=f"g{k % 2}",
                           name=f"g_{sb}_{k}")
            h, gg = k // 2, k % 2
            c0 = reg_start[(sb, k)] // 16
            for p0 in range(0, ncol, GCAP):
                pn = min(GCAP, ncol - p0)
                nidx = pn * P
                nc.gpsimd.dma_gather(
                    g[:, p0:p0 + pn, :],
                    table_h[h][gg * BANK:(gg + 1) * BANK, :],
                    idx_s[:, c0 + p0 * 8:c0 + p0 * 8 + nidx // 16],
                    nidx, nidx, D)
            return g

        def do_record(ri, k, j, t, st, sp, G, accs):
            # per-record S build: tensor_scalar is_equal hits the DVE 4x
            # fast mode (a wide tensor_tensor with a 0-stride broadcast
            # operand does not)
            S = spool.tile([P, P], kdt, tag="s")
            nc.vector.tensor_scalar(
                out=S[:], in0=iota_s[:, :P],
                scalar1=dkT_s[:, ri:ri + 1],
                scalar2=None, op0=mybir.AluOpType.is_equal)
            nc.tensor.matmul(
                out=accs[t], lhsT=S[:],
                rhs=G[k][:, j, :], start=st, stop=sp)

        def do_records(recs, G, accs, rec_base):
            for i, rec in enumerate(recs):
                _sb, k, j, t, lo, hi, st, sp = rec
                do_record(rec_base + i, k, j, t, st, sp, G, accs)

        nA = sum(len(lay["recsA"][s]) for s in range(NSB))
        if _STAGED:
            # ---- stage A: banks 0,1 (half 0) for every super-block
            rec_base = 0
            for sb in range(NSB):
                ts = list(range(sb * SBB, min((sb + 1) * SBB, NBLK)))
                G = {k: do_gather(sb, k) for k in (0, 1)}
                accs = alloc_accs(sb, ts, "accA")
                recs = lay["recsA"][sb]
                do_records(recs, G, accs, rec_base)
                rec_base += len(recs)
                for t in ts:
                    if hasA[t]:
                        nc.scalar.activation(
                            out=partA[:, t * D:(t + 1) * D], in_=accs[t],
                            func=mybir.ActivationFunctionType.Copy)
            # ---- stage B: banks 2,3 (half 1) + self-loop + epilogue.
            # Fully sequential chain per block within its PSUM bank:
            # init(partA) -> records -> identity(hl2own, stop).
            rec_base = nA
            for sb in range(NSB):
                ts = list(range(sb * SBB, min((sb + 1) * SBB, NBLK)))
                G = {k: do_gather(sb, k) for k in (2, 3)}
                accs = alloc_accs(sb, ts, "accB")
                recs = lay["recsB"][sb]
                by_t = {}
                for i, rec in enumerate(recs):
                    by_t.setdefault(rec[3], []).append((rec_base + i, rec))
                for t in ts:
                    if hasA[t]:
                        nc.tensor.matmul(out=accs[t], lhsT=ident_s[:],
                                         rhs=partA[:, t * D:(t + 1) * D],
                                         start=True, stop=False)
                    for ri, rec in by_t.get(t, []):
                        _sb, k, j, _t, lo, hi, st, sp = rec
                        do_record(ri, k, j, t, st, sp, G, accs)
                    nc.tensor.matmul(out=accs[t], lhsT=ident_s[:],
                                     rhs=hl2_slice(t),
                                     start=not hasA[t] and not hasB[t],
                                     stop=True)
                rec_base += len(recs)
                ostg = epi.tile([P, SBB * D], kdt, tag="ostg")
                for lt, t in enumerate(ts):
                    nc.scalar.activation(
                        out=ostg[:, lt * D:(lt + 1) * D], in_=accs[t],
                        func=mybir.ActivationFunctionType.Relu,
                        scale=disT_s[:, t:t + 1])
                eng = nc.sync if sb % 2 == 0 else nc.scalar
                eng.dma_start(
                    out=out[:, sb * SBB * D:(sb * SBB + len(ts)) * D],
                    in_=ostg[:, :len(ts) * D])
        else:
            # single pass per super-block; one continuous chain per block:
            # records(A) -> records(B) -> identity(hl2own, stop)
            baseA = 0
            baseB = nA
            for sb in range(NSB):
                ts = list(range(sb * SBB, min((sb + 1) * SBB, NBLK)))
                G = {k: do_gather(sb, k) for k in range(NBANKS)}
                accs = alloc_accs(sb, ts, "acc")
                recsA = lay["recsA"][sb]
                recsB = lay["recsB"][sb]
                by_t = {}
                for i, rec in enumerate(recsA):
                    by_t.setdefault(rec[3], []).append((baseA + i, rec))
                for i, rec in enumerate(recsB):
                    by_t.setdefault(rec[3], []).append((baseB + i, rec))
                for t in ts:
                    first = True
                    for ri, rec in by_t.get(t, []):
                        _sb, k, j, _t, lo, hi, _st, _sp = rec
                        do_record(ri, k, j, t, first, False, G, accs)
                        first = False
                    nc.tensor.matmul(out=accs[t], lhsT=ident_s[:],
                                     rhs=hl2_slice(t),
                                     start=first, stop=True)
                baseA += len(recsA)
                baseB += len(recsB)
                ostg = epi.tile([P, SBB * D], kdt, tag="ostg")
                for lt, t in enumerate(ts):
                    nc.scalar.activation(
                        out=ostg[:, lt * D:(lt + 1) * D], in_=accs[t],
                        func=mybir.ActivationFunctionType.Relu,
                        scale=disT_s[:, t:t + 1])
                eng = nc.sync if sb % 2 == 0 else nc.scalar
                eng.dma_start(
                    out=out[:, sb * SBB * D:(sb * SBB + len(ts)) * D],
                    in_=ostg[:, :len(ts) * D])


_PB_USED = None  # legacy hook for bench.py; now caches layout key


def kernel(H, edge_index, W, b):
    from concourse.bass_utils import run_bass_kernel_spmd

    global _PB_USED
    in_maps, meta = _host_prep(H, edge_index, W, b)
    key = (meta["nrec"], meta["nslots"], _STAGED, _SHARED_AG)
    _PB_USED = key
    if key not in _NC_CACHE:
        _NC_CACHE[key] = _build_nc(meta)
    nc = _NC_CACHE[key]

    res = run_bass_kernel_spmd(nc, in_maps, list(range(NCORES)))
    outs = []
    for c in range(NCORES):
        o = np.asarray(res.results[c]["out"]).reshape(P, NBLK, D)
        outs.append(o.transpose(1, 0, 2).reshape(NPC, D).astype(np.float32))
    out = np.concatenate(outs, axis=0)
    return np.ascontiguousarray(out[:N])
